# revision 37
# baseline (speedup 1.0000x reference)
"""Trainium2 Bass kernel for nn_MultiHeadRot (RoPE multi-head causal attention).

Sharding: tensor-parallel over heads — each of the 8 cores owns 2 of the 16
heads: it computes the QKV projection for its head pair, RoPE, causal
attention, and a partial output projection against its 128-column slice of
w_out. The host sums the 8 partial outputs (the TP all-reduce happens at
gather time).

Layout choices (per core):
  - Activations live feature-major on chip: xT/q/k/ctx are [d_model|128, tokens].
  - RoPE keeps the reference's interleaved pair layout: rotate_half is a
    swap of adjacent partitions, done with a single DVE stream_shuffle
    (mask swaps 2i<->2i+1 within each 32-partition group); the pair sign
    is folded into a host-prepared signed sin table. No rotate matmul.
  - Attention computes transposed scores S^T = K_blk^T Q_chunk ([k=128, q=512]
    per block), exp on ScalarE into bf16 probs (both heads in one strided
    activation on diagonal blocks), multiplicative staircase masks on the
    causal-diagonal blocks, and P V via a [k,65] stationary operand whose
    65th column of ones accumulates the softmax denominators alongside
    the context.
  - Normalization: reciprocal_approx_fast on the denominator row, gpsimd
    partition-broadcast, then one multiply when copying ctx out of PSUM.
All matmuls run in bf16 (fp32 PSUM accumulation); softmax runs in fp32.
"""

import sys

for _p in ("/opt/trn_rl_repo", "/opt/pypackages"):
    if _p not in sys.path:
        sys.path.insert(0, _p)

import numpy as np
import ml_dtypes

BF16 = ml_dtypes.bfloat16
F8E4 = ml_dtypes.float8_e4m3fn

B, S, D, NH, HD = 4, 2048, 1024, 16, 64
T = B * S
NCORES = 8
CH = 512          # token chunk (free dim) for projections / attention q-chunks
NCHUNK = T // CH  # 16

# stream_shuffle mask: swap adjacent partitions within each 32-group
SWAP_MASK = [i ^ 1 for i in range(32)]

_PROGRAM = None


def _build_program():
    import concourse.bass as bass
    import concourse.mybir as mybir
    import concourse.tile as tile
    from concourse import bacc
    from concourse.bass import ds

    dt = mybir.dt
    AF = mybir.ActivationFunctionType

    nc = bacc.Bacc("TRN2", debug=False)

    xT_d = nc.dram_tensor("xT", [D, T], dt.bfloat16, kind="ExternalInput")
    wq_d = nc.dram_tensor("wqT", [D, 128], dt.bfloat16, kind="ExternalInput")
    wk_d = nc.dram_tensor("wkT", [D, 128], dt.bfloat16, kind="ExternalInput")
    wv_d = nc.dram_tensor("wvT", [D, 128], dt.bfloat16, kind="ExternalInput")
    wo_d = nc.dram_tensor("woT", [128, D], dt.bfloat16, kind="ExternalInput")
    cos_d = nc.dram_tensor("cosT", [128, S], dt.bfloat16, kind="ExternalInput")
    sin_d = nc.dram_tensor("sinT", [128, S], dt.bfloat16, kind="ExternalInput")
    mask_d = nc.dram_tensor("masks", [128, 128], dt.bfloat16, kind="ExternalInput")
    out_d = nc.dram_tensor("out", [T, D], dt.bfloat16, kind="ExternalOutput")

    with tile.TileContext(nc) as tc:
        with (
            tc.tile_pool(name="const", bufs=1) as cp,
            tc.tile_pool(name="persist", bufs=1) as pp,
            tc.tile_pool(name="xs", bufs=3) as xp,
            tc.tile_pool(name="psA", bufs=2, space="PSUM") as psA,
            tc.tile_pool(name="spsum", bufs=2, space="PSUM") as sps,
            tc.tile_pool(name="ctxpsum", bufs=2, space="PSUM") as cps,
            tc.tile_pool(name="sb1", bufs=3) as sb1,
            tc.tile_pool(name="sb2", bufs=4) as sb2,
            tc.tile_pool(name="sbt", bufs=3) as sbt,
            tc.tile_pool(name="sb3", bufs=12) as sb3,
        ):
            wq_sb = cp.tile([128, 8, 128], dt.bfloat16, tag="wq")
            wk_sb = cp.tile([128, 8, 128], dt.bfloat16, tag="wk")
            wv_sb = cp.tile([128, 8, 128], dt.bfloat16, tag="wv")
            wo_sb = cp.tile([128, D], dt.bfloat16, tag="wo")
            cos_sb = cp.tile([128, S], dt.bfloat16, tag="cos")
            sin_sb = cp.tile([128, S], dt.bfloat16, tag="sin")
            mask_sb = cp.tile([128, 1, 128], dt.bfloat16, tag="mask")
            warm_sb = cp.tile([1, 8], dt.float32, tag="warm")

            nc.sync.dma_start(wq_sb[:], wq_d.ap().rearrange("(i p) o -> p i o", p=128))
            nc.sync.dma_start(wk_sb[:], wk_d.ap().rearrange("(i p) o -> p i o", p=128))
            nc.sync.dma_start(wv_sb[:], wv_d.ap().rearrange("(i p) o -> p i o", p=128))
            nc.gpsimd.dma_start(wo_sb[:], wo_d.ap())
            nc.gpsimd.dma_start(cos_sb[:], cos_d.ap())
            nc.gpsimd.dma_start(sin_sb[:], sin_d.ap())
            nc.gpsimd.dma_start(mask_sb[:, 0], mask_d.ap())

            # warm the ACT exp table set before the first real activation
            nc.vector.memset(warm_sb[:], 0.0)
            nc.scalar.activation(warm_sb[:], warm_sb[:], AF.Exp)

            # PE warm-up: dummy matmuls fill the initial DMA-wait window so
            # the HAM clock gate reaches full rate before real work arrives
            dmy_w = cp.tile([128, 128], dt.bfloat16, tag="dmyw")
            dmy_x = cp.tile([128, CH], dt.bfloat16, tag="dmyx")
            nc.vector.memset(dmy_w[:], 0.0)
            nc.vector.memset(dmy_x[:], 0.0)
            dmy_ps = psA.tile([128, CH], dt.float32, tag="a", name="dmyps")
            NWARM = 44
            for i in range(NWARM):
                nc.tensor.matmul(
                    dmy_ps[:], dmy_w[:], dmy_x[:],
                    start=(i == 0), stop=(i == NWARM - 1),
                )

            qrot = pp.tile([128, NCHUNK, CH], dt.bfloat16, tag="qrot")
            krot = pp.tile([128, NCHUNK, CH], dt.bfloat16, tag="krot")
            vsb = pp.tile([128, T // 128, 2, 65], dt.bfloat16, tag="vsb")
            ctx = pp.tile([128, NCHUNK, CH], dt.bfloat16, tag="ctx")

            nc.vector.memset(vsb[:, :, :, 64:65], 1.0)

            xT_ap = xT_d.ap().rearrange("(i p) t -> p i t", p=128)
            out_ap = out_d.ap().rearrange("(tb p) e -> tb p e", p=128)

            xts = {}

            def proj_qk_items(ch, split=False):
                """QKV projection + RoPE for chunk ch as small work items."""
                xt = xp.tile([128, 8, CH], dt.bfloat16, tag="xt", name="xt")
                xts[ch] = xt
                if split:
                    # per-slice DMAs on two queues so the first accumulating
                    # matmuls can start before the whole chunk has landed
                    for i in range(8):
                        eng = nc.sync if i % 2 == 0 else nc.gpsimd
                        eng.dma_start(xt[:, i], xT_ap[:, i, ds(ch * CH, CH)])
                else:
                    nc.sync.dma_start(xt[:], xT_ap[:, :, ds(ch * CH, CH)])
                s0 = (ch % (S // CH)) * CH

                def qk_item(w_sb, dst):
                    ps = psA.tile([128, CH], dt.float32, tag="a", name="projps")
                    for i in range(8):
                        nc.tensor.matmul(
                            ps[:], w_sb[:, i], xt[:, i],
                            start=(i == 0), stop=(i == 7),
                        )
                    rot_f = sb1.tile([128, CH], dt.float32, tag="rotf", name="rotf")
                    nc.vector.stream_shuffle(rot_f[:], ps[:], SWAP_MASK)
                    t1 = sb1.tile([128, CH], dt.bfloat16, tag="t1", name="t1")
                    t2 = sb1.tile([128, CH], dt.bfloat16, tag="t2", name="t2")
                    nc.vector.tensor_mul(t1[:], ps[:], cos_sb[:, ds(s0, CH)])
                    nc.vector.tensor_mul(t2[:], rot_f[:], sin_sb[:, ds(s0, CH)])
                    nc.vector.tensor_add(dst[:, ch], t1[:], t2[:])

                yield lambda: qk_item(wq_sb, qrot)
                yield lambda: qk_item(wk_sb, krot)

            def proj_v_items(ch):
                xt = xts[ch]

                def v_item(j):
                    tb = ch * 4 + j
                    pv = psA.tile([128, 2, 64], dt.float32, tag="a", name="vps")
                    for i in range(8):
                        nc.tensor.matmul(
                            pv[:], xt[:, i, ds(j * 128, 128)], wv_sb[:, i],
                            start=(i == 0), stop=(i == 7),
                        )
                    nc.vector.tensor_copy(vsb[:, tb, :, 0:64], pv[:])

                for j in range(4):
                    yield lambda j=j: v_item(j)

            def attn_units(b, ci):
                """Software-pipelined: scores(kj+1) is emitted before PV(kj)
                so the PV's wait on exp(kj) hides behind the next score
                matmuls. One score item + one PV item per key block kj, plus
                a tail unit."""
                qch = b * (S // CH) + ci
                nkb = 4 * ci + 4
                ctxp = [cps.tile([65, CH], dt.float32, tag="ctx",
                                  name=f"ctxp{h}") for h in range(2)]
                prs = {}

                def geom(kj):
                    diag = kj >= 4 * ci
                    off = 128 * (kj - 4 * ci) if diag else 0
                    return diag, off, CH - off

                def score_item(kj):
                    diag, off, n = geom(kj)
                    sp = sps.tile([128, 2, CH], dt.float32, tag="s", name="sp")
                    for h in range(2):
                        hs = h * 64
                        k_ap = krot[hs:hs + 64, b * 4 + kj // 4,
                                    ds((kj % 4) * 128, 128)]
                        nc.tensor.matmul(
                            sp[:, h, ds(off, n)], k_ap,
                            qrot[hs:hs + 64, qch, ds(off, n)],
                            start=True, stop=True,
                        )
                    pr = sb2.tile([128, 2, CH], dt.bfloat16, tag="pr", name="pr")
                    prs[kj] = pr
                    nc.scalar.activation(
                        pr[:, :, ds(off, n)], sp[:, :, ds(off, n)], AF.Exp
                    )
                    if diag:
                        nc.vector.tensor_mul(
                            pr[:, :, ds(off, 128)], pr[:, :, ds(off, 128)],
                            mask_sb[:].broadcast_to([128, 2, 128]),
                        )

                def pv_item(kj):
                    diag, off, n = geom(kj)
                    pr = prs.pop(kj)
                    for h in range(2):
                        nc.tensor.matmul(
                            ctxp[h][:, ds(off, n)],
                            vsb[:, b * 16 + kj, h],
                            pr[:, h, ds(off, n)],
                            start=(kj == 0), stop=(kj == nkb - 1),
                        )

                def tail():
                    for h in range(2):
                        # copy the denominator row to partition 0 first:
                        # reciprocal_approx_fast mis-reads nonzero-base APs
                        dsb = sbt.tile([1, CH], dt.float32, tag="d", name="dsb")
                        nc.vector.tensor_copy(dsb[:], ctxp[h][64:65, :])
                        rsb = sbt.tile([1, CH], dt.float32, tag="r", name="rsb")
                        nc.vector.reciprocal_approx_fast(rsb[:], dsb[:])
                        rbc = sbt.tile([64, CH], dt.float32, tag="rbc", name="rbc")
                        nc.gpsimd.partition_broadcast(rbc[:], rsb[:])
                        if h == 0:
                            nc.vector.tensor_mul(
                                ctx[0:64, qch, :], ctxp[h][0:64, :], rbc[:]
                            )
                        else:
                            cb = sbt.tile([64, CH], dt.bfloat16, tag="cb", name="cb")
                            nc.vector.tensor_mul(cb[:], ctxp[h][0:64, :], rbc[:])
                            nc.sync.dma_start(ctx[64:128, qch, :], cb[:])

                yield lambda: score_item(0)
                for kj in range(nkb - 1):
                    yield lambda kj=kj: score_item(kj + 1)
                    yield lambda kj=kj: pv_item(kj)
                yield lambda: pv_item(nkb - 1)
                yield tail

            def outproj_items(b, ci, last=False):
                qch = b * (S // CH) + ci
                for jb in range(4):
                    def item(jb=jb):
                        lhs = ctx[:, qch, ds(jb * 128, 128)]
                        for e in range(2):
                            op = psA.tile([128, CH], dt.float32, tag="a", name="ops")
                            nc.tensor.matmul(
                                op[:], lhs, wo_sb[:, ds(e * CH, CH)],
                                start=True, stop=True,
                            )
                            osb = sb3.tile([128, CH], dt.bfloat16, tag="osb",
                                            name="osb")
                            # in the final flush alternate engines so the
                            # copies pipeline two-wide
                            if (e == 0 and jb != 3) or (last and e == 0):
                                nc.scalar.copy(osb[:], op[:])
                            else:
                                nc.vector.tensor_copy(osb[:], op[:])
                            nc.sync.dma_start(
                                out_ap[b * 16 + ci * 4 + jb, :, ds(e * CH, CH)],
                                osb[:],
                            )
                    yield item

            # Software pipeline with explicit emission interleaving:
            # while emitting attention units for (b, ci), spread filler items
            # (next chunk's projection + previous chunk's output projection)
            # between them so the PE stream stays dense during exp waits.
            chunks = [(b, ci) for b in range(B) for ci in range(S // CH)]

            def emit_interleaved():
                pending = []  # filler thunks
                # prologue: chunk 0+1 q/k projection and chunk 0 v projection
                # (xt DMA latency is ~7us, so keep input prefetch two deep);
                # v(ch) is only needed by attention(ch), so its items are
                # deferred a chunk to keep late-kernel PE filler available.
                for it in proj_qk_items(0, split=True):
                    it()
                for it in proj_v_items(0):
                    it()
                for it in proj_qk_items(1, split=True):
                    it()
                for idx, (b, ci) in enumerate(chunks):
                    units = list(attn_units(b, ci))
                    if idx + 2 < len(chunks):
                        pending.extend(proj_qk_items(idx + 2))
                    if idx + 1 < len(chunks):
                        pending.extend(proj_v_items(idx + 1))
                    if idx > 0:
                        pending.extend(outproj_items(*chunks[idx - 1]))
                    nf = len(pending)
                    nu = len(units)
                    taken = 0
                    for k, u in enumerate(units):
                        u()
                        want = (k + 1) * nf // nu
                        while taken < want:
                            pending[taken]()
                            taken += 1
                    pending = pending[taken:]
                for it in pending:
                    it()
                for it in outproj_items(*chunks[-1], last=True):
                    it()

            emit_interleaved()

    nc.compile()
    return nc


def _get_program():
    global _PROGRAM
    if _PROGRAM is None:
        _PROGRAM = _build_program()
    return _PROGRAM


def _host_prep(x, w_qkv, w_out):
    """Build the per-core and shared device input arrays."""
    inv_freq = 1.0 / (10000.0 ** (np.arange(0, HD, 2, dtype=np.float64) / HD))
    ang = np.arange(S, dtype=np.float64)[None, :] * inv_freq[:, None]  # [32, S]
    cos64 = np.repeat(np.cos(ang), 2, axis=0)                          # [64, S]
    # rotate_half pairs: out[2i] = -x[2i+1], out[2i+1] = x[2i]; the shuffle
    # swaps without negating, so bake the sign into sin rows 2i.
    sin64 = np.repeat(np.sin(ang), 2, axis=0)
    sin64[0::2] *= -1.0
    cosT = np.ascontiguousarray(np.concatenate([cos64, cos64], 0)).astype(BF16)
    sinT = np.ascontiguousarray(np.concatenate([sin64, sin64], 0)).astype(BF16)

    xT = np.ascontiguousarray(x.reshape(T, D).T).astype(BF16)

    ql = np.arange(128)[None, :]
    kl = np.arange(128)[:, None]
    masks = (ql >= kl).astype(BF16)  # [128, 128] staircase for diag blocks

    shared = dict(xT=xT, cosT=cosT, sinT=sinT, masks=masks)
    per_core = []
    for c in range(NCORES):
        h0 = 2 * c
        rows = np.arange(HD)
        rows_q = np.concatenate([h * HD + rows for h in (h0, h0 + 1)])
        rows_k = np.concatenate([D + h * HD + rows for h in (h0, h0 + 1)])
        rows_v = np.concatenate([2 * D + h * HD + rows for h in (h0, h0 + 1)])
        per_core.append(
            dict(
                wqT=np.ascontiguousarray((w_qkv[rows_q, :] * 0.125).T).astype(BF16),
                wkT=np.ascontiguousarray(w_qkv[rows_k, :].T).astype(BF16),
                wvT=np.ascontiguousarray(w_qkv[rows_v, :].T).astype(BF16),
                woT=np.ascontiguousarray(w_out[:, c * 128:(c + 1) * 128].T).astype(BF16),
            )
        )
    return shared, per_core


def run(x, w_qkv, w_out, trace=False):
    """Run the sharded kernel; returns (out [B,S,D] f32, BassKernelResults)."""
    from concourse import bass_utils

    x = np.asarray(x, dtype=np.float32)
    w_qkv = np.asarray(w_qkv, dtype=np.float32)
    w_out = np.asarray(w_out, dtype=np.float32)

    shared, per_core = _host_prep(x, w_qkv, w_out)
    nc = _get_program()

    in_maps = [{**shared, **per_core[c]} for c in range(NCORES)]
    last_exc = None
    for _attempt in range(3):
        try:
            res = bass_utils.run_bass_kernel_spmd(
                nc, in_maps, core_ids=list(range(NCORES)), trace=trace
            )
            break
        except Exception as e:  # transient NRT/axon failures — retry
            last_exc = e
    else:
        raise last_exc
    out = res.results[0]["out"].astype(np.float32)
    for c in range(1, NCORES):
        out = out + res.results[c]["out"].astype(np.float32)
    return out.reshape(B, S, D), res


def kernel(x, w_qkv, w_out, src_mask=None, src_padding=None, is_causal=1):
    out, _ = run(x, w_qkv, w_out)
    return out


# revision 41
# speedup vs baseline: 1.0327x; 1.0327x over previous
"""Trainium2 Bass kernel for nn_MultiHeadRot (RoPE multi-head causal attention).

Sharding: tensor-parallel over heads — each of the 8 cores owns 2 of the 16
heads: it computes the QKV projection for its head pair, RoPE, causal
attention, and a partial output projection against its 128-column slice of
w_out. The host sums the 8 partial outputs (the TP all-reduce happens at
gather time).

Layout choices (per core):
  - Activations live feature-major on chip: xT/q/k/ctx are [d_model|128, tokens].
  - RoPE keeps the reference's interleaved pair layout: rotate_half is a
    swap of adjacent partitions, done with a single DVE stream_shuffle
    (mask swaps 2i<->2i+1 within each 32-partition group); the pair sign
    is folded into a host-prepared signed sin table. No rotate matmul.
  - Attention computes transposed scores S^T = K_blk^T Q_chunk ([k=128, q=512]
    per block), exp on ScalarE into bf16 probs (both heads in one strided
    activation on diagonal blocks), multiplicative staircase masks on the
    causal-diagonal blocks, and P V via a [k,65] stationary operand whose
    65th column of ones accumulates the softmax denominators alongside
    the context.
  - Normalization: reciprocal_approx_fast on the denominator row, gpsimd
    partition-broadcast, then one multiply when copying ctx out of PSUM.
All matmuls run in bf16 (fp32 PSUM accumulation); softmax runs in fp32.
"""

import sys

for _p in ("/opt/trn_rl_repo", "/opt/pypackages"):
    if _p not in sys.path:
        sys.path.insert(0, _p)

import numpy as np
import ml_dtypes

BF16 = ml_dtypes.bfloat16
F8E4 = ml_dtypes.float8_e4m3fn

B, S, D, NH, HD = 4, 2048, 1024, 16, 64
T = B * S
NCORES = 8
CH = 512          # token chunk (free dim) for projections / attention q-chunks
NCHUNK = T // CH  # 16

# stream_shuffle mask: swap adjacent partitions within each 32-group
SWAP_MASK = [i ^ 1 for i in range(32)]

_PROGRAM = None


def _build_program():
    import concourse.bass as bass
    import concourse.mybir as mybir
    import concourse.tile as tile
    from concourse import bacc
    from concourse.bass import ds

    dt = mybir.dt
    AF = mybir.ActivationFunctionType

    nc = bacc.Bacc("TRN2", debug=False)

    xT_d = nc.dram_tensor("xT", [D, T], dt.bfloat16, kind="ExternalInput")
    wq_d = nc.dram_tensor("wqT", [D, 128], dt.bfloat16, kind="ExternalInput")
    wk_d = nc.dram_tensor("wkT", [D, 128], dt.bfloat16, kind="ExternalInput")
    wv_d = nc.dram_tensor("wvT", [D, 128], dt.bfloat16, kind="ExternalInput")
    wo_d = nc.dram_tensor("woT", [128, D], dt.bfloat16, kind="ExternalInput")
    cos_d = nc.dram_tensor("cosT", [128, S], dt.bfloat16, kind="ExternalInput")
    sin_d = nc.dram_tensor("sinT", [128, S], dt.bfloat16, kind="ExternalInput")
    mask_d = nc.dram_tensor("masks", [128, 128], dt.bfloat16, kind="ExternalInput")
    out_d = nc.dram_tensor("out", [T, D], dt.bfloat16, kind="ExternalOutput")

    with tile.TileContext(nc) as tc:
        with (
            tc.tile_pool(name="const", bufs=1) as cp,
            tc.tile_pool(name="persist", bufs=1) as pp,
            tc.tile_pool(name="xs", bufs=3) as xp,
            tc.tile_pool(name="psA", bufs=2, space="PSUM") as psA,
            tc.tile_pool(name="spsum", bufs=2, space="PSUM") as sps,
            tc.tile_pool(name="ctxpsum", bufs=2, space="PSUM") as cps,
            tc.tile_pool(name="sb1", bufs=3) as sb1,
            tc.tile_pool(name="sb2", bufs=4) as sb2,
            tc.tile_pool(name="sbt", bufs=3) as sbt,
            tc.tile_pool(name="sb3", bufs=12) as sb3,
        ):
            wq_sb = cp.tile([128, 8, 128], dt.bfloat16, tag="wq")
            wk_sb = cp.tile([128, 8, 128], dt.bfloat16, tag="wk")
            wv_sb = cp.tile([128, 8, 128], dt.bfloat16, tag="wv")
            wo_sb = cp.tile([128, D], dt.bfloat16, tag="wo")
            cos_sb = cp.tile([128, S], dt.bfloat16, tag="cos")
            sin_sb = cp.tile([128, S], dt.bfloat16, tag="sin")
            mask_sb = cp.tile([128, 1, 128], dt.bfloat16, tag="mask")
            warm_sb = cp.tile([1, 8], dt.float32, tag="warm")

            xT_ap = xT_d.ap().rearrange("(i p) t -> p i t", p=128)
            # chunk-0 input slices first: they gate the first real matmul
            # and DMA completion latency here is ~9us
            xt0 = xp.tile([128, 8, CH], dt.bfloat16, tag="xt", name="xt")
            for i in range(8):
                eng = nc.sync if i % 2 == 0 else nc.gpsimd
                eng.dma_start(xt0[:, i], xT_ap[:, i, ds(0, CH)])
            nc.sync.dma_start(wq_sb[:], wq_d.ap().rearrange("(i p) o -> p i o", p=128))
            nc.sync.dma_start(wk_sb[:], wk_d.ap().rearrange("(i p) o -> p i o", p=128))
            nc.sync.dma_start(wv_sb[:], wv_d.ap().rearrange("(i p) o -> p i o", p=128))
            nc.gpsimd.dma_start(cos_sb[:], cos_d.ap())
            nc.gpsimd.dma_start(sin_sb[:], sin_d.ap())
            nc.gpsimd.dma_start(wo_sb[:], wo_d.ap())
            nc.gpsimd.dma_start(mask_sb[:, 0], mask_d.ap())

            # warm the ACT exp table set before the first real activation
            nc.vector.memset(warm_sb[:], 0.0)
            nc.scalar.activation(warm_sb[:], warm_sb[:], AF.Exp)

            # PE warm-up: dummy matmuls fill the initial DMA-wait window so
            # the HAM clock gate reaches full rate before real work arrives
            dmy_w = cp.tile([128, 128], dt.bfloat16, tag="dmyw")
            dmy_x = cp.tile([128, CH], dt.bfloat16, tag="dmyx")
            nc.vector.memset(dmy_w[:], 0.0)
            nc.vector.memset(dmy_x[:], 0.0)
            dmy_ps = psA.tile([128, CH], dt.float32, tag="a", name="dmyps")
            NWARM = 38
            for i in range(NWARM):
                nc.tensor.matmul(
                    dmy_ps[:], dmy_w[:], dmy_x[:],
                    start=(i == 0), stop=(i == NWARM - 1),
                )

            qrot = pp.tile([128, NCHUNK, CH], dt.bfloat16, tag="qrot")
            krot = pp.tile([128, NCHUNK, CH], dt.bfloat16, tag="krot")
            vsb = pp.tile([128, T // 128, 2, 65], dt.bfloat16, tag="vsb")
            ctx = pp.tile([128, NCHUNK, CH], dt.bfloat16, tag="ctx")

            nc.vector.memset(vsb[:, :, :, 64:65], 1.0)

            out_ap = out_d.ap().rearrange("(tb p) (g e) -> tb p g e", p=128, g=2)

            xts = {0: xt0}

            def proj_qk_items(ch, split=False):
                """QKV projection + RoPE for chunk ch as small work items."""
                if ch in xts:
                    xt = xts[ch]
                else:
                    xt = xp.tile([128, 8, CH], dt.bfloat16, tag="xt", name="xt")
                    xts[ch] = xt
                    if split:
                        # per-slice DMAs on two queues so the first matmuls
                        # can start before the whole chunk has landed
                        for i in range(8):
                            eng = nc.sync if i % 2 == 0 else nc.gpsimd
                            eng.dma_start(xt[:, i], xT_ap[:, i, ds(ch * CH, CH)])
                    else:
                        nc.sync.dma_start(xt[:], xT_ap[:, :, ds(ch * CH, CH)])
                s0 = (ch % (S // CH)) * CH

                def qk_item(w_sb, dst):
                    ps = psA.tile([128, CH], dt.float32, tag="a", name="projps")
                    for i in range(8):
                        nc.tensor.matmul(
                            ps[:], w_sb[:, i], xt[:, i],
                            start=(i == 0), stop=(i == 7),
                        )
                    rot_f = sb1.tile([128, CH], dt.float32, tag="rotf", name="rotf")
                    nc.vector.stream_shuffle(rot_f[:], ps[:], SWAP_MASK)
                    t1 = sb1.tile([128, CH], dt.bfloat16, tag="t1", name="t1")
                    t2 = sb1.tile([128, CH], dt.bfloat16, tag="t2", name="t2")
                    nc.vector.tensor_mul(t1[:], ps[:], cos_sb[:, ds(s0, CH)])
                    nc.vector.tensor_mul(t2[:], rot_f[:], sin_sb[:, ds(s0, CH)])
                    nc.vector.tensor_add(dst[:, ch], t1[:], t2[:])

                yield lambda: qk_item(wq_sb, qrot)
                yield lambda: qk_item(wk_sb, krot)

            def proj_v_items(ch):
                xt = xts[ch]

                def v_item(j):
                    tb = ch * 4 + j
                    pv = psA.tile([128, 2, 64], dt.float32, tag="a", name="vps")
                    for i in range(8):
                        nc.tensor.matmul(
                            pv[:], xt[:, i, ds(j * 128, 128)], wv_sb[:, i],
                            start=(i == 0), stop=(i == 7),
                        )
                    nc.vector.tensor_copy(vsb[:, tb, :, 0:64], pv[:])

                for j in range(4):
                    yield lambda j=j: v_item(j)

            def attn_units(b, ci):
                """Software-pipelined: scores(kj+1) is emitted before PV(kj)
                so the PV's wait on exp(kj) hides behind the next score
                matmuls. One score item + one PV item per key block kj, plus
                a tail unit."""
                qch = b * (S // CH) + ci
                nkb = 4 * ci + 4
                ctxp = [cps.tile([65, CH], dt.float32, tag="ctx",
                                  name=f"ctxp{h}") for h in range(2)]
                prs = {}

                def geom(kj):
                    diag = kj >= 4 * ci
                    off = 128 * (kj - 4 * ci) if diag else 0
                    return diag, off, CH - off

                def score_item(kj):
                    diag, off, n = geom(kj)
                    sp = sps.tile([128, 2, CH], dt.float32, tag="s", name="sp")
                    for h in range(2):
                        hs = h * 64
                        k_ap = krot[hs:hs + 64, b * 4 + kj // 4,
                                    ds((kj % 4) * 128, 128)]
                        nc.tensor.matmul(
                            sp[:, h, ds(off, n)], k_ap,
                            qrot[hs:hs + 64, qch, ds(off, n)],
                            start=True, stop=True,
                        )
                    pr = sb2.tile([128, 2, CH], dt.bfloat16, tag="pr", name="pr")
                    prs[kj] = pr
                    nc.scalar.activation(
                        pr[:, :, ds(off, n)], sp[:, :, ds(off, n)], AF.Exp
                    )
                    if diag:
                        nc.vector.tensor_mul(
                            pr[:, :, ds(off, 128)], pr[:, :, ds(off, 128)],
                            mask_sb[:].broadcast_to([128, 2, 128]),
                        )

                def pv_item(kj):
                    diag, off, n = geom(kj)
                    pr = prs.pop(kj)
                    for h in range(2):
                        nc.tensor.matmul(
                            ctxp[h][:, ds(off, n)],
                            vsb[:, b * 16 + kj, h],
                            pr[:, h, ds(off, n)],
                            start=(kj == 0), stop=(kj == nkb - 1),
                        )

                def tail():
                    for h in range(2):
                        # copy the denominator row to partition 0 first:
                        # reciprocal_approx_fast mis-reads nonzero-base APs
                        dsb = sbt.tile([1, CH], dt.float32, tag="d", name="dsb")
                        nc.vector.tensor_copy(dsb[:], ctxp[h][64:65, :])
                        rsb = sbt.tile([1, CH], dt.float32, tag="r", name="rsb")
                        nc.vector.reciprocal_approx_fast(rsb[:], dsb[:])
                        rbc = sbt.tile([64, CH], dt.float32, tag="rbc", name="rbc")
                        nc.gpsimd.partition_broadcast(rbc[:], rsb[:])
                        if h == 0:
                            nc.vector.tensor_mul(
                                ctx[0:64, qch, :], ctxp[h][0:64, :], rbc[:]
                            )
                        else:
                            cb = sbt.tile([64, CH], dt.bfloat16, tag="cb", name="cb")
                            nc.vector.tensor_mul(cb[:], ctxp[h][0:64, :], rbc[:])
                            nc.sync.dma_start(ctx[64:128, qch, :], cb[:])

                yield lambda: score_item(0)
                for kj in range(nkb - 1):
                    yield lambda kj=kj: score_item(kj + 1)
                    yield lambda kj=kj: pv_item(kj)
                yield lambda: pv_item(nkb - 1)
                yield tail

            def outproj_items(b, ci, last=False):
                qch = b * (S // CH) + ci
                for jb in range(4):
                    def item(jb=jb):
                        lhs = ctx[:, qch, ds(jb * 128, 128)]
                        osb = sb3.tile([128, 2, CH], dt.bfloat16, tag="osb",
                                        name="osb")
                        for e in range(2):
                            op = psA.tile([128, CH], dt.float32, tag="a", name="ops")
                            nc.tensor.matmul(
                                op[:], lhs, wo_sb[:, ds(e * CH, CH)],
                                start=True, stop=True,
                            )
                            # in the final flush alternate engines so the
                            # copies pipeline two-wide
                            if (e == 0 and jb != 3) or (last and e == 0):
                                nc.scalar.copy(osb[:, e], op[:])
                            else:
                                nc.vector.tensor_copy(osb[:, e], op[:])
                        nc.sync.dma_start(out_ap[b * 16 + ci * 4 + jb], osb[:])
                    yield item

            # Software pipeline with explicit emission interleaving:
            # while emitting attention units for (b, ci), spread filler items
            # (next chunk's projection + previous chunk's output projection)
            # between them so the PE stream stays dense during exp waits.
            chunks = [(b, ci) for b in range(B) for ci in range(S // CH)]

            def emit_interleaved():
                pending = []  # filler thunks
                # prologue: chunk 0+1 q/k projection and chunk 0 v projection
                # (xt DMA latency is ~7us, so keep input prefetch two deep);
                # v(ch) is only needed by attention(ch), so its items are
                # deferred a chunk to keep late-kernel PE filler available.
                for it in proj_qk_items(0, split=True):
                    it()
                for it in proj_v_items(0):
                    it()
                for it in proj_qk_items(1, split=True):
                    it()
                for idx, (b, ci) in enumerate(chunks):
                    units = list(attn_units(b, ci))
                    if idx + 2 < len(chunks):
                        pending.extend(proj_qk_items(idx + 2))
                    if idx + 1 < len(chunks):
                        pending.extend(proj_v_items(idx + 1))
                    if idx > 0:
                        pending.extend(outproj_items(*chunks[idx - 1]))
                    nf = len(pending)
                    nu = len(units)
                    taken = 0
                    for k, u in enumerate(units):
                        u()
                        want = (k + 1) * nf // nu
                        while taken < want:
                            pending[taken]()
                            taken += 1
                    pending = pending[taken:]
                for it in pending:
                    it()
                for it in outproj_items(*chunks[-1], last=True):
                    it()

            emit_interleaved()

    nc.compile()
    return nc


def _get_program():
    global _PROGRAM
    if _PROGRAM is None:
        _PROGRAM = _build_program()
    return _PROGRAM


def _host_prep(x, w_qkv, w_out):
    """Build the per-core and shared device input arrays."""
    inv_freq = 1.0 / (10000.0 ** (np.arange(0, HD, 2, dtype=np.float64) / HD))
    ang = np.arange(S, dtype=np.float64)[None, :] * inv_freq[:, None]  # [32, S]
    cos64 = np.repeat(np.cos(ang), 2, axis=0)                          # [64, S]
    # rotate_half pairs: out[2i] = -x[2i+1], out[2i+1] = x[2i]; the shuffle
    # swaps without negating, so bake the sign into sin rows 2i.
    sin64 = np.repeat(np.sin(ang), 2, axis=0)
    sin64[0::2] *= -1.0
    cosT = np.ascontiguousarray(np.concatenate([cos64, cos64], 0)).astype(BF16)
    sinT = np.ascontiguousarray(np.concatenate([sin64, sin64], 0)).astype(BF16)

    xT = np.ascontiguousarray(x.reshape(T, D).T).astype(BF16)

    ql = np.arange(128)[None, :]
    kl = np.arange(128)[:, None]
    masks = (ql >= kl).astype(BF16)  # [128, 128] staircase for diag blocks

    shared = dict(xT=xT, cosT=cosT, sinT=sinT, masks=masks)
    per_core = []
    for c in range(NCORES):
        h0 = 2 * c
        rows = np.arange(HD)
        rows_q = np.concatenate([h * HD + rows for h in (h0, h0 + 1)])
        rows_k = np.concatenate([D + h * HD + rows for h in (h0, h0 + 1)])
        rows_v = np.concatenate([2 * D + h * HD + rows for h in (h0, h0 + 1)])
        per_core.append(
            dict(
                wqT=np.ascontiguousarray((w_qkv[rows_q, :] * 0.125).T).astype(BF16),
                wkT=np.ascontiguousarray(w_qkv[rows_k, :].T).astype(BF16),
                wvT=np.ascontiguousarray(w_qkv[rows_v, :].T).astype(BF16),
                woT=np.ascontiguousarray(w_out[:, c * 128:(c + 1) * 128].T).astype(BF16),
            )
        )
    return shared, per_core


def run(x, w_qkv, w_out, trace=False):
    """Run the sharded kernel; returns (out [B,S,D] f32, BassKernelResults)."""
    from concourse import bass_utils

    x = np.asarray(x, dtype=np.float32)
    w_qkv = np.asarray(w_qkv, dtype=np.float32)
    w_out = np.asarray(w_out, dtype=np.float32)

    shared, per_core = _host_prep(x, w_qkv, w_out)
    nc = _get_program()

    in_maps = [{**shared, **per_core[c]} for c in range(NCORES)]
    last_exc = None
    for _attempt in range(3):
        try:
            res = bass_utils.run_bass_kernel_spmd(
                nc, in_maps, core_ids=list(range(NCORES)), trace=trace
            )
            break
        except Exception as e:  # transient NRT/axon failures — retry
            last_exc = e
    else:
        raise last_exc
    out = res.results[0]["out"].astype(np.float32)
    for c in range(1, NCORES):
        out = out + res.results[c]["out"].astype(np.float32)
    return out.reshape(B, S, D), res


def kernel(x, w_qkv, w_out, src_mask=None, src_padding=None, is_causal=1):
    out, _ = run(x, w_qkv, w_out)
    return out


# revision 45
# speedup vs baseline: 1.0351x; 1.0023x over previous
"""Trainium2 Bass kernel for nn_MultiHeadRot (RoPE multi-head causal attention).

Sharding: tensor-parallel over heads — each of the 8 cores owns 2 of the 16
heads: it computes the QKV projection for its head pair, RoPE, causal
attention, and a partial output projection against its 128-column slice of
w_out. The host sums the 8 partial outputs (the TP all-reduce happens at
gather time).

Layout choices (per core):
  - Activations live feature-major on chip: xT/q/k/ctx are [d_model|128, tokens].
  - RoPE keeps the reference's interleaved pair layout: rotate_half is a
    swap of adjacent partitions, done with a single DVE stream_shuffle
    (mask swaps 2i<->2i+1 within each 32-partition group); the pair sign
    is folded into a host-prepared signed sin table. No rotate matmul.
  - Attention computes transposed scores S^T = K_blk^T Q_chunk ([k=128, q=512]
    per block), exp on ScalarE into bf16 probs (both heads in one strided
    activation on diagonal blocks), multiplicative staircase masks on the
    causal-diagonal blocks, and P V via a [k,65] stationary operand whose
    65th column of ones accumulates the softmax denominators alongside
    the context.
  - Normalization: reciprocal_approx_fast on the denominator row, gpsimd
    partition-broadcast, then one multiply when copying ctx out of PSUM.
All matmuls run in bf16 (fp32 PSUM accumulation); softmax runs in fp32.
"""

import sys

for _p in ("/opt/trn_rl_repo", "/opt/pypackages"):
    if _p not in sys.path:
        sys.path.insert(0, _p)

import numpy as np
import ml_dtypes

BF16 = ml_dtypes.bfloat16
F8E4 = ml_dtypes.float8_e4m3fn

B, S, D, NH, HD = 4, 2048, 1024, 16, 64
T = B * S
NCORES = 8
CH = 512          # token chunk (free dim) for projections / attention q-chunks
NCHUNK = T // CH  # 16

# stream_shuffle mask: swap adjacent partitions within each 32-group
SWAP_MASK = [i ^ 1 for i in range(32)]

_PROGRAM = None


def _build_program():
    import concourse.bass as bass
    import concourse.mybir as mybir
    import concourse.tile as tile
    from concourse import bacc
    from concourse.bass import ds

    dt = mybir.dt
    AF = mybir.ActivationFunctionType

    nc = bacc.Bacc("TRN2", debug=False)

    xT_d = nc.dram_tensor("xT", [D, T], dt.bfloat16, kind="ExternalInput")
    wq_d = nc.dram_tensor("wqT", [D, 128], dt.bfloat16, kind="ExternalInput")
    wk_d = nc.dram_tensor("wkT", [D, 128], dt.bfloat16, kind="ExternalInput")
    wv_d = nc.dram_tensor("wvT", [D, 128], dt.bfloat16, kind="ExternalInput")
    wo_d = nc.dram_tensor("woT", [128, D], dt.bfloat16, kind="ExternalInput")
    cos_d = nc.dram_tensor("cosT", [128, S], dt.bfloat16, kind="ExternalInput")
    sin_d = nc.dram_tensor("sinT", [128, S], dt.bfloat16, kind="ExternalInput")
    mask_d = nc.dram_tensor("masks", [128, 128], dt.bfloat16, kind="ExternalInput")
    out_d = nc.dram_tensor("out", [T, D], dt.bfloat16, kind="ExternalOutput")

    with tile.TileContext(nc) as tc:
        with (
            tc.tile_pool(name="const", bufs=1) as cp,
            tc.tile_pool(name="persist", bufs=1) as pp,
            tc.tile_pool(name="xs", bufs=5) as xp,
            tc.tile_pool(name="psA", bufs=2, space="PSUM") as psA,
            tc.tile_pool(name="spsum", bufs=2, space="PSUM") as sps,
            tc.tile_pool(name="ctxpsum", bufs=2, space="PSUM") as cps,
            tc.tile_pool(name="sb1", bufs=3) as sb1,
            tc.tile_pool(name="sb2", bufs=6) as sb2,
            tc.tile_pool(name="sbt", bufs=3) as sbt,
            tc.tile_pool(name="sb3", bufs=12) as sb3,
        ):
            wq_sb = cp.tile([128, 8, 128], dt.bfloat16, tag="wq")
            wk_sb = cp.tile([128, 8, 128], dt.bfloat16, tag="wk")
            wv_sb = cp.tile([128, 8, 128], dt.bfloat16, tag="wv")
            wo_sb = cp.tile([128, D], dt.bfloat16, tag="wo")
            cos_sb = cp.tile([128, S], dt.bfloat16, tag="cos")
            sin_sb = cp.tile([128, S], dt.bfloat16, tag="sin")
            mask_sb = cp.tile([128, 1, 128], dt.bfloat16, tag="mask")
            warm_sb = cp.tile([1, 8], dt.float32, tag="warm")

            xT_ap = xT_d.ap().rearrange("(i p) t -> p i t", p=128)
            # chunk-0 input slices first: they gate the first real matmul
            # and DMA completion latency here is ~9us
            xt0 = xp.tile([128, 8, CH], dt.bfloat16, tag="xt", name="xt")
            for i in range(8):
                eng = nc.sync if i % 2 == 0 else nc.gpsimd
                eng.dma_start(xt0[:, i], xT_ap[:, i, ds(0, CH)])
            nc.sync.dma_start(wq_sb[:], wq_d.ap().rearrange("(i p) o -> p i o", p=128))
            nc.sync.dma_start(wk_sb[:], wk_d.ap().rearrange("(i p) o -> p i o", p=128))
            nc.sync.dma_start(wv_sb[:], wv_d.ap().rearrange("(i p) o -> p i o", p=128))
            nc.gpsimd.dma_start(cos_sb[:], cos_d.ap())
            nc.gpsimd.dma_start(sin_sb[:], sin_d.ap())
            nc.gpsimd.dma_start(wo_sb[:], wo_d.ap())
            nc.gpsimd.dma_start(mask_sb[:, 0], mask_d.ap())

            # warm the ACT exp table set before the first real activation
            nc.vector.memset(warm_sb[:], 0.0)
            nc.scalar.activation(warm_sb[:], warm_sb[:], AF.Exp)

            # PE warm-up: dummy matmuls fill the initial DMA-wait window so
            # the HAM clock gate reaches full rate before real work arrives
            dmy_w = cp.tile([128, 128], dt.bfloat16, tag="dmyw")
            dmy_x = cp.tile([128, CH], dt.bfloat16, tag="dmyx")
            nc.vector.memset(dmy_w[:], 0.0)
            nc.vector.memset(dmy_x[:], 0.0)
            dmy_ps = psA.tile([128, CH], dt.float32, tag="a", name="dmyps")
            NWARM = 38
            for i in range(NWARM):
                nc.tensor.matmul(
                    dmy_ps[:], dmy_w[:], dmy_x[:],
                    start=(i == 0), stop=(i == NWARM - 1),
                )

            qrot = pp.tile([128, NCHUNK, CH], dt.bfloat16, tag="qrot")
            krot = pp.tile([128, NCHUNK, CH], dt.bfloat16, tag="krot")
            vsb = pp.tile([128, T // 128, 2, 65], dt.bfloat16, tag="vsb")
            ctx = pp.tile([128, NCHUNK, CH], dt.bfloat16, tag="ctx")

            nc.vector.memset(vsb[:, :, :, 64:65], 1.0)

            out_ap = out_d.ap().rearrange("(tb p) (g e) -> tb p g e", p=128, g=2)

            xts = {0: xt0}

            def proj_qk_items(ch, split=False):
                """QKV projection + RoPE for chunk ch as small work items."""
                if ch in xts:
                    xt = xts[ch]
                else:
                    xt = xp.tile([128, 8, CH], dt.bfloat16, tag="xt", name="xt")
                    xts[ch] = xt
                    if split:
                        # per-slice DMAs on two queues so the first matmuls
                        # can start before the whole chunk has landed
                        for i in range(8):
                            eng = nc.sync if i % 2 == 0 else nc.gpsimd
                            eng.dma_start(xt[:, i], xT_ap[:, i, ds(ch * CH, CH)])
                    else:
                        nc.sync.dma_start(xt[:], xT_ap[:, :, ds(ch * CH, CH)])
                s0 = (ch % (S // CH)) * CH

                def qk_item(w_sb, dst):
                    ps = psA.tile([128, CH], dt.float32, tag="a", name="projps")
                    for i in range(8):
                        nc.tensor.matmul(
                            ps[:], w_sb[:, i], xt[:, i],
                            start=(i == 0), stop=(i == 7),
                        )
                    rot_f = sb1.tile([128, CH], dt.float32, tag="rotf", name="rotf")
                    nc.vector.stream_shuffle(rot_f[:], ps[:], SWAP_MASK)
                    t1 = sb1.tile([128, CH], dt.bfloat16, tag="t1", name="t1")
                    t2 = sb1.tile([128, CH], dt.bfloat16, tag="t2", name="t2")
                    nc.vector.tensor_mul(t1[:], ps[:], cos_sb[:, ds(s0, CH)])
                    nc.vector.tensor_mul(t2[:], rot_f[:], sin_sb[:, ds(s0, CH)])
                    nc.vector.tensor_add(dst[:, ch], t1[:], t2[:])

                yield lambda: qk_item(wq_sb, qrot)
                yield lambda: qk_item(wk_sb, krot)

            def proj_v_items(ch):
                xt = xts[ch]

                def v_item(j):
                    tb = ch * 4 + j
                    pv = psA.tile([128, 2, 64], dt.float32, tag="a", name="vps")
                    for i in range(8):
                        nc.tensor.matmul(
                            pv[:], xt[:, i, ds(j * 128, 128)], wv_sb[:, i],
                            start=(i == 0), stop=(i == 7),
                        )
                    nc.vector.tensor_copy(vsb[:, tb, :, 0:64], pv[:])

                for j in range(4):
                    yield lambda j=j: v_item(j)

            def attn_units(b, ci):
                """Software-pipelined: scores(kj+1) is emitted before PV(kj)
                so the PV's wait on exp(kj) hides behind the next score
                matmuls. One score item + one PV item per key block kj, plus
                a tail unit."""
                qch = b * (S // CH) + ci
                nkb = 4 * ci + 4
                ctxp = [cps.tile([65, CH], dt.float32, tag="ctx",
                                  name=f"ctxp{h}") for h in range(2)]
                prs = {}

                def geom(kj):
                    diag = kj >= 4 * ci
                    off = 128 * (kj - 4 * ci) if diag else 0
                    return diag, off, CH - off

                def score_item(kj):
                    diag, off, n = geom(kj)
                    sp = sps.tile([128, 2, CH], dt.float32, tag="s", name="sp")
                    for h in range(2):
                        hs = h * 64
                        k_ap = krot[hs:hs + 64, b * 4 + kj // 4,
                                    ds((kj % 4) * 128, 128)]
                        nc.tensor.matmul(
                            sp[:, h, ds(off, n)], k_ap,
                            qrot[hs:hs + 64, qch, ds(off, n)],
                            start=True, stop=True,
                        )
                    pr = sb2.tile([128, 2, CH], dt.bfloat16, tag="pr", name="pr")
                    prs[kj] = pr
                    nc.scalar.activation(
                        pr[:, :, ds(off, n)], sp[:, :, ds(off, n)], AF.Exp
                    )
                    if diag:
                        nc.vector.tensor_mul(
                            pr[:, :, ds(off, 128)], pr[:, :, ds(off, 128)],
                            mask_sb[:].broadcast_to([128, 2, 128]),
                        )

                def pv_item(kj):
                    diag, off, n = geom(kj)
                    pr = prs.pop(kj)
                    for h in range(2):
                        nc.tensor.matmul(
                            ctxp[h][:, ds(off, n)],
                            vsb[:, b * 16 + kj, h],
                            pr[:, h, ds(off, n)],
                            start=(kj == 0), stop=(kj == nkb - 1),
                        )

                def tail():
                    for h in range(2):
                        # copy the denominator row to partition 0 first:
                        # reciprocal_approx_fast mis-reads nonzero-base APs
                        dsb = sbt.tile([1, CH], dt.float32, tag="d", name="dsb")
                        nc.vector.tensor_copy(dsb[:], ctxp[h][64:65, :])
                        rsb = sbt.tile([1, CH], dt.float32, tag="r", name="rsb")
                        nc.vector.reciprocal_approx_fast(rsb[:], dsb[:])
                        rbc = sbt.tile([64, CH], dt.float32, tag="rbc", name="rbc")
                        nc.gpsimd.partition_broadcast(rbc[:], rsb[:])
                        if h == 0:
                            nc.vector.tensor_mul(
                                ctx[0:64, qch, :], ctxp[h][0:64, :], rbc[:]
                            )
                        else:
                            cb = sbt.tile([64, CH], dt.bfloat16, tag="cb", name="cb")
                            nc.vector.tensor_mul(cb[:], ctxp[h][0:64, :], rbc[:])
                            nc.sync.dma_start(ctx[64:128, qch, :], cb[:])

                yield lambda: score_item(0)
                for kj in range(nkb - 1):
                    yield lambda kj=kj: score_item(kj + 1)
                    yield lambda kj=kj: pv_item(kj)
                yield lambda: pv_item(nkb - 1)
                yield tail

            def outproj_items(b, ci, last=False):
                qch = b * (S // CH) + ci
                for jb in range(4):
                    def item(jb=jb):
                        lhs = ctx[:, qch, ds(jb * 128, 128)]
                        osb = sb3.tile([128, 2, CH], dt.bfloat16, tag="osb",
                                        name="osb")
                        for e in range(2):
                            op = psA.tile([128, CH], dt.float32, tag="a", name="ops")
                            nc.tensor.matmul(
                                op[:], lhs, wo_sb[:, ds(e * CH, CH)],
                                start=True, stop=True,
                            )
                            # in the final flush alternate engines so the
                            # copies pipeline two-wide
                            if (e == 0 and jb != 3) or (last and e == 0):
                                nc.scalar.copy(osb[:, e], op[:])
                            else:
                                nc.vector.tensor_copy(osb[:, e], op[:])
                        nc.sync.dma_start(out_ap[b * 16 + ci * 4 + jb], osb[:])
                    yield item

            # Software pipeline with explicit emission interleaving:
            # while emitting attention units for (b, ci), spread filler items
            # (next chunk's projection + previous chunk's output projection)
            # between them so the PE stream stays dense during exp waits.
            chunks = [(b, ci) for b in range(B) for ci in range(S // CH)]

            def emit_interleaved():
                pending = []  # filler thunks
                # prologue: q/k projection for pair 0 and 1's chunks, v for
                # pair 0 (v(ch) is only needed by attention(ch), so later v
                # items stay deferred as pair-level filler).
                for it in proj_qk_items(0, split=True):
                    it()
                for it in proj_v_items(0):
                    it()
                for it in proj_qk_items(1, split=True):
                    it()
                for idx, (b, ci) in enumerate(chunks):
                    units = list(attn_units(b, ci))
                    if idx + 2 < len(chunks):
                        pending.extend(proj_qk_items(idx + 2))
                    if idx + 1 < len(chunks):
                        pending.extend(proj_v_items(idx + 1))
                    if idx > 0:
                        pending.extend(outproj_items(*chunks[idx - 1]))
                    nf = len(pending)
                    nu = len(units)
                    taken = 0
                    for k, u in enumerate(units):
                        u()
                        want = (k + 1) * nf // nu
                        while taken < want:
                            pending[taken]()
                            taken += 1
                    pending = pending[taken:]
                for it in pending:
                    it()
                for it in outproj_items(*chunks[-1], last=True):
                    it()

            emit_interleaved()

    nc.compile()
    return nc


def _get_program():
    global _PROGRAM
    if _PROGRAM is None:
        _PROGRAM = _build_program()
    return _PROGRAM


def _host_prep(x, w_qkv, w_out):
    """Build the per-core and shared device input arrays."""
    inv_freq = 1.0 / (10000.0 ** (np.arange(0, HD, 2, dtype=np.float64) / HD))
    ang = np.arange(S, dtype=np.float64)[None, :] * inv_freq[:, None]  # [32, S]
    cos64 = np.repeat(np.cos(ang), 2, axis=0)                          # [64, S]
    # rotate_half pairs: out[2i] = -x[2i+1], out[2i+1] = x[2i]; the shuffle
    # swaps without negating, so bake the sign into sin rows 2i.
    sin64 = np.repeat(np.sin(ang), 2, axis=0)
    sin64[0::2] *= -1.0
    cosT = np.ascontiguousarray(np.concatenate([cos64, cos64], 0)).astype(BF16)
    sinT = np.ascontiguousarray(np.concatenate([sin64, sin64], 0)).astype(BF16)

    xT = np.ascontiguousarray(x.reshape(T, D).T).astype(BF16)

    ql = np.arange(128)[None, :]
    kl = np.arange(128)[:, None]
    masks = (ql >= kl).astype(BF16)  # [128, 128] staircase for diag blocks

    shared = dict(xT=xT, cosT=cosT, sinT=sinT, masks=masks)
    per_core = []
    for c in range(NCORES):
        h0 = 2 * c
        rows = np.arange(HD)
        rows_q = np.concatenate([h * HD + rows for h in (h0, h0 + 1)])
        rows_k = np.concatenate([D + h * HD + rows for h in (h0, h0 + 1)])
        rows_v = np.concatenate([2 * D + h * HD + rows for h in (h0, h0 + 1)])
        per_core.append(
            dict(
                wqT=np.ascontiguousarray((w_qkv[rows_q, :] * 0.125).T).astype(BF16),
                wkT=np.ascontiguousarray(w_qkv[rows_k, :].T).astype(BF16),
                wvT=np.ascontiguousarray(w_qkv[rows_v, :].T).astype(BF16),
                woT=np.ascontiguousarray(w_out[:, c * 128:(c + 1) * 128].T).astype(BF16),
            )
        )
    return shared, per_core


def run(x, w_qkv, w_out, trace=False):
    """Run the sharded kernel; returns (out [B,S,D] f32, BassKernelResults)."""
    from concourse import bass_utils

    x = np.asarray(x, dtype=np.float32)
    w_qkv = np.asarray(w_qkv, dtype=np.float32)
    w_out = np.asarray(w_out, dtype=np.float32)

    shared, per_core = _host_prep(x, w_qkv, w_out)
    nc = _get_program()

    in_maps = [{**shared, **per_core[c]} for c in range(NCORES)]
    last_exc = None
    for _attempt in range(3):
        try:
            res = bass_utils.run_bass_kernel_spmd(
                nc, in_maps, core_ids=list(range(NCORES)), trace=trace
            )
            break
        except Exception as e:  # transient NRT/axon failures — retry
            last_exc = e
    else:
        raise last_exc
    out = res.results[0]["out"].astype(np.float32)
    for c in range(1, NCORES):
        out = out + res.results[c]["out"].astype(np.float32)
    return out.reshape(B, S, D), res


def kernel(x, w_qkv, w_out, src_mask=None, src_padding=None, is_causal=1):
    out, _ = run(x, w_qkv, w_out)
    return out
